# revision 3
# baseline (speedup 1.0000x reference)
"""DenoiseNet loss kernel for 8x Trainium2 NeuronCores (Bass/Tile).

Data-parallel over the batch dim: core b handles cloud b entirely on-chip.
Per core:
  KNN1: 128 sampled queries vs 8192 noisy points (top-32 via DVE max8 rounds)
  gather neighbor coords (indirect DMA), build transposed query tiles
  KNN2: 4096 neighbor points vs 8192 clean points (top-4 via DVE max8)
  score MLP + ground score + squared-error partials, all fused on-chip.
Scores use s = q.r - |r|^2/2 (monotone in -d2 per query row), computed with
fp32 PE matmuls against an augmented [x;y;z;-|r|^2/2] ref layout.
"""
import numpy as np

import concourse.bacc as bacc
import concourse.bass as bass
import concourse.mybir as mybir
import concourse.tile as tile
from concourse.bass_utils import run_bass_kernel_spmd
from concourse.masks import make_identity

B, N, M, P = 8, 8192, 8192, 128
K1, K2 = 32, 4
FD, H = 128, 128
NCHUNK = 16          # 8192 / 512
CW = 512             # matmul moving chunk width
NEG = -3.0e38

F32 = mybir.dt.float32
U32 = mybir.dt.uint32
RELU = mybir.ActivationFunctionType.Relu
SQUARE = mybir.ActivationFunctionType.Square
ALU = mybir.AluOpType

_cache = {}


def _build():
    nc = bacc.Bacc("TRN2", target_bir_lowering=False, debug=False,
                   enable_asserts=False, num_devices=8)

    refN = nc.dram_tensor("refN", [4, N], F32, kind="ExternalInput")
    refC = nc.dram_tensor("refC", [4, M], F32, kind="ExternalInput")
    qaugT = nc.dram_tensor("qaugT", [4, P], F32, kind="ExternalInput")
    noisyN = nc.dram_tensor("noisyN", [N, 3], F32, kind="ExternalInput")
    cleanN = nc.dram_tensor("cleanN", [M, 3], F32, kind="ExternalInput")
    w1 = nc.dram_tensor("w1", [3, H], F32, kind="ExternalInput")
    w1n = nc.dram_tensor("w1n", [3, H], F32, kind="ExternalInput")
    w2 = nc.dram_tensor("w2", [FD, H], F32, kind="ExternalInput")
    wf = nc.dram_tensor("wf", [3, FD], F32, kind="ExternalInput")
    w3 = nc.dram_tensor("w3", [H, 3], F32, kind="ExternalInput")
    loss_out = nc.dram_tensor("loss_out", [P, K1], F32, kind="ExternalOutput")

    with tile.TileContext(nc) as tc:
        with (
            tc.tile_pool(name="big", bufs=2) as big,      # s rows (P x 8192)
            tc.tile_pool(name="cons", bufs=1) as cons,    # constants
            tc.tile_pool(name="sm", bufs=2) as sm,        # small per-tile
            tc.tile_pool(name="ps_mm", bufs=4, space="PSUM") as ps_mm,
            tc.tile_pool(name="ps_a", bufs=2, space="PSUM") as ps_a,
            tc.tile_pool(name="ps_b", bufs=2, space="PSUM") as ps_b,
        ):
            # ---- constants in ----
            refN_sb = cons.tile([4, N], F32)
            nc.sync.dma_start(refN_sb[:], refN[:])
            refC_sb = cons.tile([4, M], F32)
            nc.sync.dma_start(refC_sb[:], refC[:])
            q_sb = cons.tile([4, P], F32)
            nc.sync.dma_start(q_sb[:], qaugT[:])
            w1_sb = cons.tile([3, H], F32)
            nc.sync.dma_start(w1_sb[:], w1[:])
            w1n_sb = cons.tile([3, H], F32)
            nc.sync.dma_start(w1n_sb[:], w1n[:])
            w2_sb = cons.tile([FD, H], F32)
            nc.sync.dma_start(w2_sb[:], w2[:])
            wf_sb = cons.tile([3, FD], F32)
            nc.sync.dma_start(wf_sb[:], wf[:])
            w3_sb = cons.tile([H, 3], F32)
            nc.sync.dma_start(w3_sb[:], w3[:])
            ident = cons.tile([P, P], F32)
            make_identity(nc, ident[:])

            # featT[h,p] = sum_d wf[d,h] q[p,d]
            featT_ps = ps_a.tile([P, P], F32, tag="mlp")
            nc.tensor.matmul(featT_ps[:], wf_sb[:], q_sb[0:3, :], start=True, stop=True)
            featT_sb = cons.tile([P, P], F32)
            nc.scalar.copy(featT_sb[:], featT_ps[:])

            # ---- KNN1: s1 = qaug.T @ refN ----
            s1 = big.tile([P, N], F32, tag="s")
            for c in range(NCHUNK):
                pt = ps_mm.tile([P, CW], F32, tag="mm")
                nc.tensor.matmul(pt[:], q_sb[:], refN_sb[:, CW * c:CW * (c + 1)],
                                 start=True, stop=True)
                nc.scalar.copy(s1[:, CW * c:CW * (c + 1)], pt[:])

            v32 = cons.tile([P, K1], F32)
            i32 = cons.tile([P, K1], U32)
            work = big.tile([P, N], F32, tag="s")
            cur = s1
            for r in range(4):
                nc.vector.max(v32[:, 8 * r:8 * r + 8], cur[:])
                if r < 3:
                    nc.vector.match_replace(work[:], v32[:, 8 * r:8 * r + 8],
                                            cur[:], NEG)
                    cur = work
            for r in range(4):
                nc.vector.max_index(i32[:, 8 * r:8 * r + 8],
                                    v32[:, 8 * r:8 * r + 8], s1[:])

            # gather f into stride-4 layout with 1.0 in slot 3
            f_nat4 = cons.tile([P, 4 * K1], F32)
            nc.vector.memset(f_nat4[:], 1.0)
            for k in range(K1):
                nc.gpsimd.indirect_dma_start(
                    out=f_nat4[:, 4 * k:4 * k + 3],
                    out_offset=None,
                    in_=noisyN[:],
                    in_offset=bass.IndirectOffsetOnAxis(ap=i32[:, k:k + 1], axis=0),
                )
            fT_ps = ps_a.tile([P, P], F32, tag="mlp")
            nc.tensor.transpose(fT_ps[:], f_nat4[:], ident[:])
            fT_sb = cons.tile([P, P], F32)
            nc.scalar.copy(fT_sb[:], fT_ps[:])
            faug = cons.tile([4, K1 * P], F32)
            for k in range(K1):
                nc.gpsimd.dma_start(faug[:, P * k:P * (k + 1)],
                                    fT_sb[4 * k:4 * (k + 1), :])

            loss32 = cons.tile([P, K1], F32)

            # ---- KNN2 + MLP per neighbor-rank tile ----
            for k in range(K1):
                fa = faug[:, P * k:P * (k + 1)]
                s2 = big.tile([P, M], F32, tag="s")
                for c in range(NCHUNK):
                    pt = ps_mm.tile([P, CW], F32, tag="mm")
                    nc.tensor.matmul(pt[:], fa, refC_sb[:, CW * c:CW * (c + 1)],
                                     start=True, stop=True)
                    nc.scalar.copy(s2[:, CW * c:CW * (c + 1)], pt[:])

                v8 = sm.tile([P, 8], F32)
                nc.vector.max(v8[:], s2[:])
                i8 = sm.tile([P, 8], U32)
                nc.vector.max_index(i8[:], v8[:], s2[:])

                nn = sm.tile([P, 3 * K2], F32)
                for j in range(K2):
                    nc.gpsimd.indirect_dma_start(
                        out=nn[:, 3 * j:3 * j + 3],
                        out_offset=None,
                        in_=cleanN[:],
                        in_offset=bass.IndirectOffsetOnAxis(ap=i8[:, j:j + 1], axis=0),
                    )

                # hT = relu(W1^T f - W1^T q + W2^T feat), all in one PSUM group
                hT_ps = ps_a.tile([P, P], F32, tag="mlp")
                nc.tensor.matmul(hT_ps[:], w2_sb[:], featT_sb[:], start=True, stop=False)
                nc.tensor.matmul(hT_ps[:], w1n_sb[:], q_sb[0:3, :], start=False, stop=False)
                nc.tensor.matmul(hT_ps[:], w1_sb[:], fa[0:3, :], start=False, stop=True)
                hT_sb = sm.tile([P, P], F32)
                nc.scalar.activation(hT_sb[:], hT_ps[:], RELU)

                # estim[pt,d] = sum_h hT[h,pt] w3[h,d]
                est_ps = ps_b.tile([P, 3], F32, tag="est")
                nc.tensor.matmul(est_ps[:], hT_sb[:], w3_sb[:], start=True, stop=True)

                # diff = estim + f - mean_j nn_j
                nsum = sm.tile([P, 3], F32)
                nc.vector.tensor_add(nsum[:], nn[:, 0:3], nn[:, 3:6])
                nc.vector.tensor_add(nsum[:], nsum[:], nn[:, 6:9])
                nc.vector.tensor_add(nsum[:], nsum[:], nn[:, 9:12])
                t = sm.tile([P, 3], F32)
                nc.vector.tensor_add(t[:], est_ps[:], f_nat4[:, 4 * k:4 * k + 3])
                nc.vector.tensor_scalar(nsum[:], nsum[:], 0.25, None, op0=ALU.mult)
                diff = sm.tile([P, 3], F32)
                nc.vector.tensor_sub(diff[:], t[:], nsum[:])
                sq = sm.tile([P, 3], F32)
                nc.scalar.activation(sq[:], diff[:], SQUARE,
                                     accum_out=loss32[:, k:k + 1])

            nc.sync.dma_start(loss_out[:], loss32[:])

    nc.compile()
    return nc


def kernel(noisy_pc, clean_pc, sampled_idx, W_feat, W1, W2, W3):
    noisy_pc = np.ascontiguousarray(noisy_pc, dtype=np.float32)
    clean_pc = np.ascontiguousarray(clean_pc, dtype=np.float32)
    W_feat = np.ascontiguousarray(W_feat, dtype=np.float32)
    W1 = np.ascontiguousarray(W1, dtype=np.float32)
    W2 = np.ascontiguousarray(W2, dtype=np.float32)
    W3 = np.ascontiguousarray(W3, dtype=np.float32)
    idx = np.asarray(sampled_idx, dtype=np.int64)

    if "nc" not in _cache:
        _cache["nc"] = _build()
    nc = _cache["nc"]

    ones_row = np.ones((1, P), np.float32)
    w1n = (-W1).astype(np.float32)
    in_maps = []
    for b in range(B):
        nb = noisy_pc[b]                      # (N,3)
        cb = clean_pc[b]                      # (M,3)
        q = nb[idx]                           # (P,3)
        refN_b = np.concatenate(
            [nb.T, -0.5 * (nb * nb).sum(-1)[None, :]], 0).astype(np.float32)
        refC_b = np.concatenate(
            [cb.T, -0.5 * (cb * cb).sum(-1)[None, :]], 0).astype(np.float32)
        qaugT_b = np.concatenate([q.T, ones_row], 0).astype(np.float32)
        in_maps.append(dict(
            refN=np.ascontiguousarray(refN_b),
            refC=np.ascontiguousarray(refC_b),
            qaugT=np.ascontiguousarray(qaugT_b),
            noisyN=nb, cleanN=cb,
            w1=W1, w1n=w1n, w2=W2, wf=W_feat, w3=W3,
        ))

    res = run_bass_kernel_spmd(nc, in_maps, core_ids=list(range(B)))
    total = np.float64(0.0)
    for r in res.results:
        total += np.float64(r["loss_out"].sum(dtype=np.float64))
    loss = 50.0 * total / (B * P * K1)
    return np.float32(loss)
